# revision 18
# baseline (speedup 1.0000x reference)
"""Trainium2 Bass kernel for quality-weighted cosine top-5 retrieval.

Reference semantics (per query q, memory table mem [M, C], quality [M]):
    qn  = q / max(|q|, 1e-12)
    mn  = mem / max(|mem|_row, 1e-12)
    s   = (qn . mn_j) * quality_j                 (j = 0..M-1)
    top5 scores/indices of s; w = softmax(top5 scores)
    out = q + 0.5 * sum_k w_k * mem[idx_k]

Strategy (8 NeuronCores, data-parallel over queries):
  - Each core gets 1024 queries (x shard), the full memory table, full quality.
  - scores = x_raw @ (mem_row * quality/|mem|)^T via float32r matmuls (fp32
    data at 1 cycle/row).  Dropping the per-query 1/|q| factor does not change
    ranking; it is applied to the winning scores before the softmax.
  - The table streams once in column chunks; each [128, 512] table tile is
    normalized (ACT sum-of-squares + batched sqrt, GPSIMD scale) and
    PE-transposed (4 blocks into one PSUM bank, single ACT copy out).
  - Top-5 per query: DVE Max8/MaxIndex per chunk -> 8 candidates per chunk;
    candidates (value + global index) merge with a final Max8; winner indices
    recovered with one is_equal*idx + max-reduce per rank.
  - Winning rows fetched with indirect DMA; softmax-weighted sum + residual
    add run on GPSIMD.
"""

from contextlib import ExitStack

import numpy as np

import concourse.bacc as bacc
import concourse.bass as bass
import concourse.mybir as mybir
import concourse.tile as tile
from concourse.bass_utils import run_bass_kernel_spmd
from concourse.masks import make_identity

# Problem constants (hardcoded per the harness contract).
B_FULL, S_FULL, C_DIM, M_ROWS = 4, 2048, 512, 32768
N_CORES = 8
TOP_K = 5
EPS = 1e-12
P = 128  # partitions

F32 = mybir.dt.float32
F32R = mybir.dt.float32r
U32 = mybir.dt.uint32

# Exact-integer sentinel (2^24) for masked min/max index reductions.
BIG = 16777216.0


def _chunk_plan(m, m_chunk):
    """List of (base, size) column chunks covering [0, m). Sizes are
    multiples of 512 (PSUM bank) and 128 (tile rows)."""
    plan = []
    base = 0
    while base < m:
        size = min(m_chunk, m - base)
        assert size % 512 == 0, (m, m_chunk, size)
        plan.append((base, size))
        base += size
    return plan


def _retrieval_body(ctx, tc, x_ap, mem_ap, qual_ap, out_ap, q_local, m, c, m_chunk):
    nc = tc.nc
    qt_tiles = q_local // P          # query tiles of 128
    kc_chunks = c // P               # contraction chunks of 128
    t_tiles = m // P                 # total 128-row table tiles
    plan = _chunk_plan(m, m_chunk)
    w_cand = len(plan) * 8           # candidates per query

    const = ctx.enter_context(tc.tile_pool(name="const", bufs=1))
    resident = ctx.enter_context(tc.tile_pool(name="resident", bufs=1))
    tload = ctx.enter_context(tc.tile_pool(name="tload", bufs=4))
    tnorm = ctx.enter_context(tc.tile_pool(name="tnorm", bufs=4))
    small = ctx.enter_context(tc.tile_pool(name="small", bufs=8))
    ttab = ctx.enter_context(tc.tile_pool(name="ttab", bufs=2))
    fin = ctx.enter_context(tc.tile_pool(name="fin", bufs=4))
    gathp = ctx.enter_context(tc.tile_pool(name="gath", bufs=2))
    outp = ctx.enter_context(tc.tile_pool(name="outp", bufs=3))
    psum_sim = ctx.enter_context(tc.tile_pool(name="psum_sim", bufs=2, space="PSUM"))
    psum_tp = ctx.enter_context(tc.tile_pool(name="psum_tp", bufs=2, space="PSUM"))

    # ---- constants -------------------------------------------------------
    identity = const.tile([P, P], F32)
    make_identity(nc, identity)

    # quality rearranged to tile-aligned layout: qual_rt[r, t] = quality[t*128+r]
    n_vt = (t_tiles + P - 1) // P
    qual_rt = const.tile([P, n_vt * P], F32)
    qual_tp = const.tile([P, P], F32)
    qv = qual_ap.rearrange("(t r) -> t r", r=P)  # [t_tiles, 128]
    for b in range(n_vt):
        t0 = b * P
        rows = min(P, t_tiles - t0)
        if rows < P:
            nc.gpsimd.memset(qual_tp, 0.0)
        nc.sync.dma_start(out=qual_tp[:rows, :], in_=qv[t0 : t0 + rows, :])
        pt = psum_tp.tile([P, 4, P], F32)
        nc.tensor.transpose(out=pt[:, 0, :], in_=qual_tp, identity=identity)
        nc.scalar.activation(
            out=qual_rt[:, t0 : t0 + P], in_=pt[:, 0, :],
            func=mybir.ActivationFunctionType.Copy,
        )

    # ---- query prep: load x, norms, transpose ----------------------------
    xq = resident.tile([P, qt_tiles, c], F32)      # xq[r, qi, :] = x[qi*128+r, :]
    rq = resident.tile([P, qt_tiles], F32)         # 1/max(|q|, eps)
    qT = resident.tile([P, kc_chunks, q_local], F32R)  # qT[p, kc, q] = x[q, kc*128+p]
    qss = resident.tile([P, qt_tiles], F32)

    def query_prep():
        for qi in range(qt_tiles):
            nc.sync.dma_start(out=xq[:, qi, :], in_=x_ap[qi * P : (qi + 1) * P, :])
            sq = tnorm.tile([P, c], F32, tag="sqscratch")
            nc.scalar.activation(
                out=sq, in_=xq[:, qi, :],
                func=mybir.ActivationFunctionType.Square,
                accum_out=qss[:, qi : qi + 1],
            )
            pt = psum_tp.tile([P, 4, P], F32)
            for kc in range(kc_chunks):
                nc.tensor.matmul(
                    pt[:, kc, :], lhsT=xq[:, qi, kc * P : (kc + 1) * P],
                    rhs=identity, is_transpose=True,
                    start=(kc == 0), stop=(kc == kc_chunks - 1),
                )
            nc.scalar.activation(
                out=qT[:, :, qi * P : (qi + 1) * P], in_=pt,
                func=mybir.ActivationFunctionType.Copy,
            )
        qnrm = resident.tile([P, qt_tiles], F32)
        nc.scalar.activation(
            out=qnrm, in_=qss, func=mybir.ActivationFunctionType.Sqrt
        )
        nc.gpsimd.tensor_scalar_max(qnrm, qnrm, EPS)
        nc.vector.reciprocal(out=rq, in_=qnrm)

    # ---- candidate buffers ----------------------------------------------
    cand_val = resident.tile([P, qt_tiles, w_cand], F32)
    cand_idx = resident.tile([P, qt_tiles, w_cand], F32)

    # ---- main loop over table chunks ------------------------------------
    def prep_chunk(cbase, csize):
        tiles_here = csize // P
        tbase = cbase // P
        tT = ttab.tile([P, kc_chunks, m_chunk], F32R)
        for tt in range(tiles_here):
            t_glob = tbase + tt
            ttile = tload.tile([P, c], F32)
            nc.sync.dma_start(
                out=ttile, in_=mem_ap[t_glob * P : (t_glob + 1) * P, :]
            )
            sq = tnorm.tile([P, c], F32, tag="sqscratch")
            ss = small.tile([P, 1], F32, tag="ss")
            nc.scalar.activation(
                out=sq, in_=ttile,
                func=mybir.ActivationFunctionType.Square, accum_out=ss,
            )
            nrm = small.tile([P, 1], F32, tag="nrm")
            nc.scalar.activation(
                out=nrm, in_=ss, func=mybir.ActivationFunctionType.Sqrt
            )
            nc.gpsimd.tensor_scalar_max(nrm, nrm, EPS)
            rinv = small.tile([P, 1], F32, tag="rinv")
            nc.vector.reciprocal(out=rinv, in_=nrm)
            rs = small.tile([P, 1], F32, tag="rs")
            nc.gpsimd.tensor_tensor(
                out=rs, in0=rinv, in1=qual_rt[:, t_glob : t_glob + 1],
                op=mybir.AluOpType.mult,
            )
            ntile = tnorm.tile([P, c], F32, tag="ntile")
            nc.gpsimd.tensor_scalar(
                out=ntile, in0=ttile, scalar1=rs, scalar2=None,
                op0=mybir.AluOpType.mult,
            )
            pt = psum_tp.tile([P, 4, P], F32)
            for kc in range(kc_chunks):
                nc.tensor.matmul(
                    pt[:, kc, :], lhsT=ntile[:, kc * P : (kc + 1) * P], rhs=identity,
                    is_transpose=True,
                    start=(kc == 0), stop=(kc == kc_chunks - 1),
                )
            nc.scalar.activation(
                out=tT[:, :, tt * P : (tt + 1) * P], in_=pt,
                func=mybir.ActivationFunctionType.Copy,
            )
        return tT

    def scan_chunk(ch, cbase, csize, tT):
        for qi in range(qt_tiles):
            sim = psum_sim.tile([P, m_chunk], F32)
            for kc in range(kc_chunks):
                for nh in range(csize // 512):
                    nc.tensor.matmul(
                        sim[:, nh * 512 : (nh + 1) * 512],
                        lhsT=qT[:, kc, qi * P : (qi + 1) * P],
                        rhs=tT[:, kc, nh * 512 : (nh + 1) * 512],
                        start=(kc == 0),
                        stop=(kc == kc_chunks - 1),
                    )
            nc.vector.max(
                out=cand_val[:, qi, ch * 8 : ch * 8 + 8], in_=sim[:, :csize]
            )
            idx8 = small.tile([P, 8], U32, tag="idx8")
            nc.vector.max_index(
                out=idx8, in_max=cand_val[:, qi, ch * 8 : ch * 8 + 8],
                in_values=sim[:, :csize],
            )
            nc.gpsimd.tensor_scalar(
                out=cand_idx[:, qi, ch * 8 : ch * 8 + 8], in0=idx8,
                scalar1=float(cbase), scalar2=None, op0=mybir.AluOpType.add,
            )

    tT0 = prep_chunk(*plan[0])
    query_prep()
    for ch, (cbase, csize) in enumerate(plan):
        tT = tT0 if ch == 0 else prep_chunk(cbase, csize)
        scan_chunk(ch, cbase, csize, tT)

    # ---- final per-qtile: merge, softmax, gather, combine ----------------
    for qi in range(qt_tiles):
        top8 = fin.tile([P, 8], F32, tag="top8")
        nc.vector.max(out=top8, in_=cand_val[:, qi, :])

        # softmax over top-5 (scores scaled by 1/|q|), folding in the 0.5
        b0 = fin.tile([P, 1], F32, tag="b0")
        nc.gpsimd.tensor_tensor(
            out=b0, in0=top8[:, 0:1], in1=rq[:, qi : qi + 1],
            op=mybir.AluOpType.mult,
        )
        nc.gpsimd.tensor_scalar_mul(b0, b0, -1.0)
        e5 = fin.tile([P, TOP_K], F32, tag="e5")
        nc.scalar.activation(
            out=e5, in_=top8[:, :TOP_K],
            func=mybir.ActivationFunctionType.Exp,
            scale=rq[:, qi : qi + 1], bias=b0,
        )
        ssum = fin.tile([P, 1], F32, tag="ssum")
        nc.vector.reduce_sum(out=ssum, in_=e5, axis=mybir.AxisListType.X)
        rsum = fin.tile([P, 1], F32, tag="rsum")
        nc.vector.reciprocal(out=rsum, in_=ssum)
        w5 = fin.tile([P, TOP_K], F32, tag="w5")
        nc.vector.tensor_scalar(
            out=w5, in0=e5, scalar1=rsum, scalar2=0.5,
            op0=mybir.AluOpType.mult, op1=mybir.AluOpType.mult,
        )

        # winner indices: (cand_val == t_k) * cand_idx, then max-reduce.
        idx5f = fin.tile([P, TOP_K], F32, tag="idx5f")
        for k in range(TOP_K):
            stt = fin.tile([P, w_cand], F32, tag="stt")
            nc.vector.scalar_tensor_tensor(
                out=stt, in0=cand_val[:, qi, :], scalar=top8[:, k : k + 1],
                in1=cand_idx[:, qi, :],
                op0=mybir.AluOpType.is_equal, op1=mybir.AluOpType.mult,
            )
            nc.vector.tensor_reduce(
                op=mybir.AluOpType.max, out=idx5f[:, k : k + 1], in_=stt,
                axis=mybir.AxisListType.X,
            )
        idx5u = fin.tile([P, TOP_K], U32, tag="idx5u")
        nc.gpsimd.tensor_copy(out=idx5u, in_=idx5f)

        gath = gathp.tile([P, TOP_K, c], F32)
        for k in range(TOP_K):
            nc.gpsimd.indirect_dma_start(
                out=gath[:, k, :], out_offset=None,
                in_=mem_ap,
                in_offset=bass.IndirectOffsetOnAxis(ap=idx5u[:, k : k + 1], axis=0),
            )
        # out = x + sum_k w5_k * row_k   (w5 already includes the 0.5)
        acc = outp.tile([P, c], F32)
        nc.vector.scalar_tensor_tensor(
            out=acc, in0=gath[:, 0, :], scalar=w5[:, 0:1], in1=xq[:, qi, :],
            op0=mybir.AluOpType.mult, op1=mybir.AluOpType.add,
        )
        for k in range(1, TOP_K):
            nc.vector.scalar_tensor_tensor(
                out=acc, in0=gath[:, k, :], scalar=w5[:, k : k + 1], in1=acc,
                op0=mybir.AluOpType.mult, op1=mybir.AluOpType.add,
            )
        nc.sync.dma_start(out=out_ap[qi * P : (qi + 1) * P, :], in_=acc)


def build_bass_kernel(q_local, m, c, m_chunk):
    nc = bacc.Bacc("TRN2")
    x = nc.dram_tensor("x", [q_local, c], F32, kind="ExternalInput")
    mem = nc.dram_tensor("memory_mean", [m, c], F32, kind="ExternalInput")
    qual = nc.dram_tensor("memory_quality", [m], F32, kind="ExternalInput")
    out = nc.dram_tensor("out", [q_local, c], F32, kind="ExternalOutput")
    with tile.TileContext(nc) as tc, ExitStack() as ctx:
        _retrieval_body(
            ctx, tc, x.ap(), mem.ap(), qual.ap(), out.ap(), q_local, m, c, m_chunk
        )
    # Bacc.finalize runs the bacc pass pipeline (register allocation, matmul
    # wait splitting, event semaphores) required for walrus codegen.
    nc.finalize()
    return nc


_NC_CACHE = {}


def _get_nc():
    key = "full"
    if key not in _NC_CACHE:
        _NC_CACHE[key] = build_bass_kernel(
            q_local=B_FULL * S_FULL // N_CORES, m=M_ROWS, c=C_DIM, m_chunk=1536
        )
    return _NC_CACHE[key]


def kernel(x, memory_mean, memory_quality):
    x = np.asarray(x, dtype=np.float32)
    memory_mean = np.asarray(memory_mean, dtype=np.float32)
    memory_quality = np.asarray(memory_quality, dtype=np.float32)
    b, s, c = x.shape
    n = b * s
    q_local = n // N_CORES
    xf = np.ascontiguousarray(x.reshape(n, c))
    nc = _get_nc()
    in_maps = [
        {
            "x": np.ascontiguousarray(xf[i * q_local : (i + 1) * q_local]),
            "memory_mean": memory_mean,
            "memory_quality": memory_quality,
        }
        for i in range(N_CORES)
    ]
    res = run_bass_kernel_spmd(nc, in_maps, core_ids=list(range(N_CORES)))
    outs = [res.results[i]["out"] for i in range(N_CORES)]
    return np.concatenate(outs, axis=0).reshape(b, s, c).astype(np.float32)


# revision 27
# speedup vs baseline: 76.3504x; 76.3504x over previous
"""Trainium2 Bass kernel for quality-weighted cosine top-5 retrieval.

Reference semantics (per query q, memory table mem [M, C], quality [M]):
    qn  = q / max(|q|, 1e-12)
    mn  = mem / max(|mem|_row, 1e-12)
    s   = (qn . mn_j) * quality_j                 (j = 0..M-1)
    top5 scores/indices of s; w = softmax(top5 scores)
    out = q + 0.5 * sum_k w_k * mem[idx_k]

Strategy (8 NeuronCores, data-parallel over queries):
  - Each core gets 1024 queries (x shard), the full memory table, full quality.
  - scores = x_raw @ (mem_row * quality/|mem|)^T via float32r matmuls (fp32
    data at 1 cycle/row).  Dropping the per-query 1/|q| factor does not change
    ranking; it is applied to the winning scores before the softmax.
  - The table streams once in column chunks; each [128, 512] table tile is
    normalized (ACT sum-of-squares + batched sqrt, GPSIMD scale) and
    PE-transposed (4 blocks into one PSUM bank, single ACT copy out).
  - Top-5 per query: DVE Max8/MaxIndex per chunk -> 8 candidates per chunk;
    candidates (value + global index) merge with a final Max8; winner indices
    recovered with one is_equal*idx + max-reduce per rank.
  - Winning rows fetched with indirect DMA; softmax-weighted sum + residual
    add fused into DVE scalar_tensor_tensor ops.
"""

from contextlib import ExitStack

import numpy as np

import concourse.bacc as bacc
import concourse.bass as bass
import concourse.mybir as mybir
import concourse.tile as tile
from concourse.bass_utils import run_bass_kernel_spmd
from concourse.masks import make_identity

# Problem constants (hardcoded per the harness contract).
B_FULL, S_FULL, C_DIM, M_ROWS = 4, 2048, 512, 32768
N_CORES = 8
TOP_K = 5
EPS = 1e-12
P = 128  # partitions

F32 = mybir.dt.float32
F32R = mybir.dt.float32r
U32 = mybir.dt.uint32

# Exact-integer sentinel (2^24) for masked min/max index reductions.
BIG = 16777216.0


def _chunk_plan(m, m_chunk):
    """List of (base, size) column chunks covering [0, m). Sizes are
    multiples of 512 (PSUM bank) and 128 (tile rows)."""
    plan = []
    base = 0
    while base < m:
        size = min(m_chunk, m - base)
        assert size % 512 == 0, (m, m_chunk, size)
        plan.append((base, size))
        base += size
    return plan


def _retrieval_body(ctx, tc, x_ap, mem_ap, qual_ap, out_ap, q_local, m, c, m_chunk):
    nc = tc.nc
    qt_tiles = q_local // P          # query tiles of 128
    kc_chunks = c // P               # contraction chunks of 128
    t_tiles = m // P                 # total 128-row table tiles
    plan = _chunk_plan(m, m_chunk)
    w_cand = len(plan) * 8           # candidates per query

    const = ctx.enter_context(tc.tile_pool(name="const", bufs=1))
    resident = ctx.enter_context(tc.tile_pool(name="resident", bufs=1))
    tload = ctx.enter_context(tc.tile_pool(name="tload", bufs=6))
    tnorm = ctx.enter_context(tc.tile_pool(name="tnorm", bufs=6))
    small = ctx.enter_context(tc.tile_pool(name="small", bufs=8))
    ttab = ctx.enter_context(tc.tile_pool(name="ttab", bufs=2))
    fin = ctx.enter_context(tc.tile_pool(name="fin", bufs=4))
    gathp = ctx.enter_context(tc.tile_pool(name="gath", bufs=2))
    outp = ctx.enter_context(tc.tile_pool(name="outp", bufs=3))
    psum_sim = ctx.enter_context(tc.tile_pool(name="psum_sim", bufs=2, space="PSUM"))
    psum_tp = ctx.enter_context(tc.tile_pool(name="psum_tp", bufs=2, space="PSUM"))

    # ---- constants -------------------------------------------------------
    identity = const.tile([P, P], F32)
    make_identity(nc, identity)

    # quality rearranged to tile-aligned layout: qual_rt[r, t] = quality[t*128+r]
    n_vt = (t_tiles + P - 1) // P
    qual_rt = const.tile([P, n_vt * P], F32)
    qual_tp = const.tile([P, P], F32)
    qv = qual_ap.rearrange("(t r) -> t r", r=P)  # [t_tiles, 128]
    for b in range(n_vt):
        t0 = b * P
        rows = min(P, t_tiles - t0)
        if rows < P:
            nc.gpsimd.memset(qual_tp, 0.0)
        nc.sync.dma_start(out=qual_tp[:rows, :], in_=qv[t0 : t0 + rows, :])
        pt = psum_tp.tile([P, 4, P], F32)
        nc.tensor.transpose(out=pt[:, 0, :], in_=qual_tp, identity=identity)
        nc.scalar.activation(
            out=qual_rt[:, t0 : t0 + P], in_=pt[:, 0, :],
            func=mybir.ActivationFunctionType.Copy,
        )

    # ---- query prep: load x, norms, transpose ----------------------------
    xq = resident.tile([P, qt_tiles, c], F32)      # xq[r, qi, :] = x[qi*128+r, :]
    rq = resident.tile([P, qt_tiles], F32)         # 1/max(|q|, eps)
    qT = resident.tile([P, kc_chunks, q_local], F32R)  # qT[p, kc, q] = x[q, kc*128+p]
    qss = resident.tile([P, qt_tiles], F32)

    def query_prep():
        for qi in range(qt_tiles):
            nc.sync.dma_start(out=xq[:, qi, :], in_=x_ap[qi * P : (qi + 1) * P, :])
            sq = tnorm.tile([P, c], F32, tag="sqscratch")
            nc.scalar.activation(
                out=sq, in_=xq[:, qi, :],
                func=mybir.ActivationFunctionType.Square,
                accum_out=qss[:, qi : qi + 1],
            )
            pt = psum_tp.tile([P, 4, P], F32)
            for kc in range(kc_chunks):
                nc.tensor.matmul(
                    pt[:, kc, :], lhsT=xq[:, qi, kc * P : (kc + 1) * P],
                    rhs=identity, is_transpose=True,
                    start=(kc == 0), stop=(kc == kc_chunks - 1),
                )
            nc.scalar.activation(
                out=qT[:, :, qi * P : (qi + 1) * P], in_=pt,
                func=mybir.ActivationFunctionType.Copy,
            )
        qnrm = resident.tile([P, qt_tiles], F32)
        nc.scalar.activation(
            out=qnrm, in_=qss, func=mybir.ActivationFunctionType.Sqrt
        )
        nc.gpsimd.tensor_scalar_max(qnrm, qnrm, EPS)
        nc.vector.reciprocal(out=rq, in_=qnrm)

    # ---- candidate buffers ----------------------------------------------
    cand_val = resident.tile([P, qt_tiles, w_cand], F32)
    cand_idx = resident.tile([P, qt_tiles, w_cand], F32)

    # ---- main loop over table chunks ------------------------------------
    def prep_chunk(cbase, csize, first=False):
        tiles_here = csize // P
        tbase = cbase // P
        tT = ttab.tile([P, kc_chunks, m_chunk], F32R)
        for tt in range(tiles_here):
            t_glob = tbase + tt
            ttile = tload.tile([P, c], F32)
            nc.sync.dma_start(
                out=ttile, in_=mem_ap[t_glob * P : (t_glob + 1) * P, :]
            )
            sq = tnorm.tile([P, c], F32, tag="sqscratch")
            ss = small.tile([P, 1], F32, tag="ss")
            if first:
                # DVE is idle during startup; offload sum-of-squares there to
                # shorten the ACT chain that gates the first matmuls.
                nc.vector.tensor_tensor(
                    out=sq, in0=ttile, in1=ttile, op=mybir.AluOpType.mult
                )
                nc.vector.reduce_sum(out=ss, in_=sq, axis=mybir.AxisListType.X)
            else:
                nc.scalar.activation(
                    out=sq, in_=ttile,
                    func=mybir.ActivationFunctionType.Square, accum_out=ss,
                )
            nrm = small.tile([P, 1], F32, tag="nrm")
            nc.scalar.activation(
                out=nrm, in_=ss, func=mybir.ActivationFunctionType.Sqrt
            )
            nc.gpsimd.tensor_scalar_max(nrm, nrm, EPS)
            rinv = small.tile([P, 1], F32, tag="rinv")
            nc.vector.reciprocal(out=rinv, in_=nrm)
            rs = small.tile([P, 1], F32, tag="rs")
            nc.gpsimd.tensor_tensor(
                out=rs, in0=rinv, in1=qual_rt[:, t_glob : t_glob + 1],
                op=mybir.AluOpType.mult,
            )
            ntile = tnorm.tile([P, c], F32, tag="ntile")
            nc.gpsimd.tensor_scalar(
                out=ntile, in0=ttile, scalar1=rs, scalar2=None,
                op0=mybir.AluOpType.mult,
            )
            pt = psum_tp.tile([P, 4, P], F32)
            for kc in range(kc_chunks):
                nc.tensor.matmul(
                    pt[:, kc, :], lhsT=ntile[:, kc * P : (kc + 1) * P], rhs=identity,
                    is_transpose=True,
                    start=(kc == 0), stop=(kc == kc_chunks - 1),
                )
            nc.scalar.activation(
                out=tT[:, :, tt * P : (tt + 1) * P], in_=pt,
                func=mybir.ActivationFunctionType.Copy,
            )
        return tT

    def scan_chunk(ch, cbase, csize, tT):
        for qi in range(qt_tiles):
            sim = psum_sim.tile([P, m_chunk], F32)
            for kc in range(kc_chunks):
                for nh in range(csize // 512):
                    nc.tensor.matmul(
                        sim[:, nh * 512 : (nh + 1) * 512],
                        lhsT=qT[:, kc, qi * P : (qi + 1) * P],
                        rhs=tT[:, kc, nh * 512 : (nh + 1) * 512],
                        start=(kc == 0),
                        stop=(kc == kc_chunks - 1),
                    )
            nc.vector.max(
                out=cand_val[:, qi, ch * 8 : ch * 8 + 8], in_=sim[:, :csize]
            )
            idx8 = small.tile([P, 8], U32, tag="idx8")
            nc.vector.max_index(
                out=idx8, in_max=cand_val[:, qi, ch * 8 : ch * 8 + 8],
                in_values=sim[:, :csize],
            )
            nc.gpsimd.tensor_scalar(
                out=cand_idx[:, qi, ch * 8 : ch * 8 + 8], in0=idx8,
                scalar1=float(cbase), scalar2=None, op0=mybir.AluOpType.add,
            )

    tT0 = prep_chunk(*plan[0], first=True)
    query_prep()
    for ch, (cbase, csize) in enumerate(plan):
        tT = tT0 if ch == 0 else prep_chunk(cbase, csize)
        scan_chunk(ch, cbase, csize, tT)

    # ---- final per-qtile: merge, softmax, gather, combine ----------------
    for qi in range(qt_tiles):
        top8 = fin.tile([P, 8], F32, tag="top8")
        nc.vector.max(out=top8, in_=cand_val[:, qi, :])

        # softmax over top-5 (scores scaled by 1/|q|), folding in the 0.5
        b0 = fin.tile([P, 1], F32, tag="b0")
        nc.gpsimd.tensor_tensor(
            out=b0, in0=top8[:, 0:1], in1=rq[:, qi : qi + 1],
            op=mybir.AluOpType.mult,
        )
        nc.gpsimd.tensor_scalar_mul(b0, b0, -1.0)
        e5 = fin.tile([P, TOP_K], F32, tag="e5")
        nc.scalar.activation(
            out=e5, in_=top8[:, :TOP_K],
            func=mybir.ActivationFunctionType.Exp,
            scale=rq[:, qi : qi + 1], bias=b0,
        )
        ssum = fin.tile([P, 1], F32, tag="ssum")
        nc.vector.reduce_sum(out=ssum, in_=e5, axis=mybir.AxisListType.X)
        rsum = fin.tile([P, 1], F32, tag="rsum")
        nc.vector.reciprocal(out=rsum, in_=ssum)
        w5 = fin.tile([P, TOP_K], F32, tag="w5")
        nc.vector.tensor_scalar(
            out=w5, in0=e5, scalar1=rsum, scalar2=0.5,
            op0=mybir.AluOpType.mult, op1=mybir.AluOpType.mult,
        )

        # winner indices: (cand_val == t_k) * cand_idx, then max-reduce.
        idx5f = fin.tile([P, TOP_K], F32, tag="idx5f")
        for k in range(TOP_K):
            stt = fin.tile([P, w_cand], F32, tag="stt")
            nc.vector.scalar_tensor_tensor(
                out=stt, in0=cand_val[:, qi, :], scalar=top8[:, k : k + 1],
                in1=cand_idx[:, qi, :],
                op0=mybir.AluOpType.is_equal, op1=mybir.AluOpType.mult,
            )
            nc.vector.tensor_reduce(
                op=mybir.AluOpType.max, out=idx5f[:, k : k + 1], in_=stt,
                axis=mybir.AxisListType.X,
            )
        idx5u = fin.tile([P, TOP_K], U32, tag="idx5u")
        nc.gpsimd.tensor_copy(out=idx5u, in_=idx5f)

        gath = gathp.tile([P, TOP_K, c], F32)
        for k in range(TOP_K):
            nc.gpsimd.indirect_dma_start(
                out=gath[:, k, :], out_offset=None,
                in_=mem_ap,
                in_offset=bass.IndirectOffsetOnAxis(ap=idx5u[:, k : k + 1], axis=0),
            )
        # out = x + sum_k w5_k * row_k   (w5 already includes the 0.5)
        acc = outp.tile([P, c], F32)
        nc.vector.scalar_tensor_tensor(
            out=acc, in0=gath[:, 0, :], scalar=w5[:, 0:1], in1=xq[:, qi, :],
            op0=mybir.AluOpType.mult, op1=mybir.AluOpType.add,
        )
        for k in range(1, TOP_K):
            nc.vector.scalar_tensor_tensor(
                out=acc, in0=gath[:, k, :], scalar=w5[:, k : k + 1], in1=acc,
                op0=mybir.AluOpType.mult, op1=mybir.AluOpType.add,
            )
        nc.sync.dma_start(out=out_ap[qi * P : (qi + 1) * P, :], in_=acc)


def build_bass_kernel(q_local, m, c, m_chunk):
    nc = bacc.Bacc("TRN2")
    x = nc.dram_tensor("x", [q_local, c], F32, kind="ExternalInput")
    mem = nc.dram_tensor("memory_mean", [m, c], F32, kind="ExternalInput")
    qual = nc.dram_tensor("memory_quality", [m], F32, kind="ExternalInput")
    out = nc.dram_tensor("out", [q_local, c], F32, kind="ExternalOutput")
    with tile.TileContext(nc) as tc, ExitStack() as ctx:
        _retrieval_body(
            ctx, tc, x.ap(), mem.ap(), qual.ap(), out.ap(), q_local, m, c, m_chunk
        )
    # Bacc.finalize runs the bacc pass pipeline (register allocation, matmul
    # wait splitting, event semaphores) required for walrus codegen.
    nc.finalize()
    return nc


_NC_CACHE = {}


def _get_nc():
    key = "full"
    if key not in _NC_CACHE:
        _NC_CACHE[key] = build_bass_kernel(
            q_local=B_FULL * S_FULL // N_CORES, m=M_ROWS, c=C_DIM, m_chunk=1536
        )
    return _NC_CACHE[key]


def kernel(x, memory_mean, memory_quality):
    x = np.asarray(x, dtype=np.float32)
    memory_mean = np.asarray(memory_mean, dtype=np.float32)
    memory_quality = np.asarray(memory_quality, dtype=np.float32)
    b, s, c = x.shape
    n = b * s
    q_local = n // N_CORES
    xf = np.ascontiguousarray(x.reshape(n, c))
    nc = _get_nc()
    in_maps = [
        {
            "x": np.ascontiguousarray(xf[i * q_local : (i + 1) * q_local]),
            "memory_mean": memory_mean,
            "memory_quality": memory_quality,
        }
        for i in range(N_CORES)
    ]
    res = run_bass_kernel_spmd(nc, in_maps, core_ids=list(range(N_CORES)))
    outs = [res.results[i]["out"] for i in range(N_CORES)]
    return np.concatenate(outs, axis=0).reshape(b, s, c).astype(np.float32)
